# revision 1
# baseline (speedup 1.0000x reference)
"""Trainium2 kernel for nn_CandidateFinder: LSH/Wu-Manber/Trie-masked top-64
candidate retrieval.

Math: for query (b,i) and key (b,j), the pair is a candidate iff
  sig-match:  sign-pattern of query_up[3,i] equals sign-pattern of key_up[3,j]
  lsh-match:  lsh_hash(query_up[b,i]) == lsh_hash(key_up[b,j])
  inserted:   prefix-6 sign patterns of query_up[0,j] and key_up[0,j] agree
and candidates are ranked by sims = query_up[b,i] . key_up[b,j] descending.

The device kernel fuses all three masks and the similarity into a single
PE matmul per (query,key) block producing
  z = C*(sig_agreement + 2*lsh_onehot_dot + 4*inserted) + sims
with C=1024.  A pair is a candidate iff z >= T (= 70656): matched pairs give
integer mask part 70*C, best non-matched 68*C, and |sims| << C.  Ordering by
z among matched pairs equals ordering by sims.  Per query row the DVE
max/max_index instruction pair extracts the top-8 (value-descending, ties by
lower index — identical to jax.lax.top_k's stable order).  Rows with more
than 8 candidates (8th value >= T) are detected and recomputed on host; for
iid-random inputs the expected candidate count per row is ~0 (an exact
64-bit sign-pattern collision is needed), so this path never triggers in
practice.
"""

import os
import sys

for _p in ("/opt/trn_rl_repo", os.path.expanduser("~/.axon_site/_ro/trn_rl_repo")):
    if os.path.isdir(_p) and _p not in sys.path:
        sys.path.insert(0, _p)

import numpy as np

B, S, D, H = 4, 4096, 64, 16
K_MAX = 64
PREFIX_LEN = 6
LSH_BUCKETS = 64
LSH_BANDWIDTH = 4.0
NEG = np.float32(-1e30)

N_CORES = 8
QN = (B * S) // N_CORES  # 2048 query rows per core
KN = S                   # 4096 keys (replicated)

C_SCALE = 1024.0
W_LSH = 2.0
W_INS = 4.0
# matched: 70*C + sims ; best unmatched: 68*C + sims ; |sims| <= ~260
THRESH = 69.0 * C_SCALE

_CACHE = {}


def _build_nc(reps=1):
    import concourse.bacc as bacc
    import concourse.mybir as mybir
    from concourse import masks
    from concourse.tile import TileContext

    dt = mybir.dt
    AF = mybir.ActivationFunctionType
    OP = mybir.AluOpType

    nc = bacc.Bacc("TRN2", target_bir_lowering=False, debug=False,
                   num_devices=N_CORES)

    qb = nc.dram_tensor("qb", [QN, D], dt.float32, kind="ExternalInput")
    q3 = nc.dram_tensor("q3", [QN, D], dt.float32, kind="ExternalInput")
    kb = nc.dram_tensor("kb", [KN, D], dt.float32, kind="ExternalInput")
    k3 = nc.dram_tensor("k3", [KN, D], dt.float32, kind="ExternalInput")
    wmq = nc.dram_tensor("wmq", [KN, PREFIX_LEN], dt.float32, kind="ExternalInput")
    wmk = nc.dram_tensor("wmk", [KN, PREFIX_LEN], dt.float32, kind="ExternalInput")
    lshw = nc.dram_tensor("lshw", [D, H], dt.float32, kind="ExternalInput")

    v8_out = nc.dram_tensor("v8", [QN, 16], dt.float32, kind="ExternalOutput")
    i8_out = nc.dram_tensor("i8", [QN, 16], dt.uint32, kind="ExternalOutput")

    MAGIC = 12582912.0  # 1.5 * 2**23 : float32 round-to-nearest-int magic
    QT = QN // 128      # 16 query tiles
    KC = KN // 128      # 32 key chunks
    QC = QN // 128      # 16 query chunks

    with TileContext(nc) as tc:
        with (
            tc.tile_pool(name="const", bufs=1) as cst,
            tc.tile_pool(name="feat", bufs=1) as feat,
            tc.tile_pool(name="hsb", bufs=6) as hsb,
            tc.tile_pool(name="eqp", bufs=2) as eqp,
            tc.tile_pool(name="sgtmp", bufs=2) as sgtmp,
            tc.tile_pool(name="prep", bufs=2, space="PSUM") as prep,
        ):
            ident = cst.tile([128, 128], dt.float32)
            masks.make_identity(nc, ident[:])
            w_sb = cst.tile([D, H], dt.float32)
            nc.sync.dma_start(w_sb[:], lshw[:])
            w_bf = cst.tile([D, H], dt.bfloat16)
            nc.scalar.activation(w_bf[:], w_sb[:], AF.Copy)
            ones_16x64 = cst.tile([H, 64], dt.float32)
            nc.vector.memset(ones_16x64[:], 1.0)
            ones6 = cst.tile([PREFIX_LEN, 1], dt.float32)
            nc.vector.memset(ones6[:], 1.0)
            iota_i = cst.tile([64, 1], dt.int32)
            nc.gpsimd.iota(iota_i[:], pattern=[[1, 1]], base=0, channel_multiplier=1)
            iota_f = cst.tile([64, 1], dt.float32)
            nc.scalar.activation(iota_f[:], iota_i[:], AF.Copy)

            # staged inputs: [128, nchunk*64]; chunk j col-block = tokens j*128..j*128+127
            kb_st = feat.tile([128, KC * D], dt.float32)
            k3_st = feat.tile([128, KC * D], dt.float32)
            qb_st = feat.tile([128, QC * D], dt.float32)
            q3_st = feat.tile([128, QC * D], dt.float32)
            wmq_st = feat.tile([128, KC * PREFIX_LEN], dt.float32)
            wmk_st = feat.tile([128, KC * PREFIX_LEN], dt.float32)

            def stage_half(dst, src, d, h, nh):
                ntok = (KC // nh) * 128 if dst in (kb_st, k3_st) else 0
                c0 = h * (ntok // 128) * d
                nc.sync.dma_start(
                    dst[:, c0:c0 + (ntok // 128) * d]
                    .rearrange("p (n d) -> p n d", d=d),
                    src[h * ntok:(h + 1) * ntok].rearrange("(n p) d -> p n d", p=128))

            def stage(dst, src, d):
                nc.sync.dma_start(dst[:].rearrange("p (n d) -> p n d", d=d),
                                  src[:].rearrange("(n p) d -> p n d", p=128))

            # persistent feature tensors
            fk1 = feat.tile([128, KN], dt.bfloat16)   # [0:64] sig(k3) ±1 | [64:128] onehot(kh)
            fk2 = feat.tile([65, KN], dt.bfloat16)    # [0:64] raw kb | [64] 4096*ins
            wq1 = feat.tile([128, QN], dt.bfloat16)   # [0:64] C*sig(q3) | [64:128] 2048*onehot(qh)
            wq2 = feat.tile([65, QN], dt.bfloat16)    # [0:64] raw qb | [64] 1.0
            kbt = feat.tile([D, KN], dt.float32)      # kb^T fp32 (lsh matmul rhs)
            qbt = feat.tile([D, QN], dt.float32)      # qb^T fp32
            sg_q0 = feat.tile([PREFIX_LEN, KN], dt.float32)
            sg_k0 = feat.tile([PREFIX_LEN, KN], dt.float32)
            v8_acc = feat.tile([128, QT * 16], dt.float32)
            i8_acc = feat.tile([128, QT * 16], dt.uint32)

            nc.gpsimd.memset(wq2[64:65, :], 1.0)

            def transpose_group(st, g):
                pt = prep.tile([D, 1024], dt.float32, tag="ps")
                for j in range(8):
                    c = g * 8 + j
                    nc.tensor.transpose(pt[:, j * 128:(j + 1) * 128],
                                        st[:, c * D:(c + 1) * D], ident[:])
                return pt

            def hash_group(xt, onehot_dst, scale2, g, floor_on_dve=False):
                cols = slice(g * 1024, (g + 1) * 1024)
                ph = prep.tile([H, 1024], dt.float32, tag="ps")
                for hh in range(2):
                    c0 = g * 1024 + hh * 512
                    nc.tensor.matmul(ph[:, hh * 512:(hh + 1) * 512], w_sb[:],
                                     xt[:, c0:c0 + 512], start=True, stop=True)
                # floor(proj/4) via round-to-nearest magic
                if floor_on_dve:
                    c1 = hsb.tile([H, 1024], dt.float32, tag="h")
                    nc.vector.tensor_scalar(c1[:], ph[:], 1.0 / LSH_BANDWIDTH, -0.5,
                                            OP.mult, OP.add)
                    c3 = hsb.tile([H, 1024], dt.float32, tag="h")
                    nc.vector.tensor_scalar(c3[:], c1[:], MAGIC, -MAGIC,
                                            OP.add, OP.add)
                else:
                    c1 = hsb.tile([H, 1024], dt.float32, tag="h")
                    nc.scalar.activation(c1[:], ph[:], AF.Copy,
                                         scale=1.0 / LSH_BANDWIDTH, bias=-0.5)
                    c2 = hsb.tile([H, 1024], dt.float32, tag="h")
                    nc.scalar.activation(c2[:], c1[:], AF.Copy, bias=MAGIC)
                    c3 = hsb.tile([H, 1024], dt.float32, tag="h")
                    nc.scalar.activation(c3[:], c2[:], AF.Copy, bias=-MAGIC)
                # fused sum+broadcast: [64, 1024] of per-token code sums
                pb = prep.tile([64, 1024], dt.float32, tag="ps")
                for hh in range(2):
                    nc.tensor.matmul(pb[:, hh * 512:(hh + 1) * 512], ones_16x64[:],
                                     c3[:, hh * 512:(hh + 1) * 512],
                                     start=True, stop=True)
                si = hsb.tile([64, 1024], dt.int32, tag="h")
                nc.scalar.activation(si[:], pb[:], AF.Copy)
                hi = hsb.tile([64, 1024], dt.int32, tag="h")
                nc.vector.tensor_scalar(hi[:], si[:], 63, None, OP.bitwise_and)
                hf = hsb.tile([64, 1024], dt.float32, tag="h")
                nc.scalar.activation(hf[:], hi[:], AF.Copy)
                if scale2 is None:
                    nc.vector.tensor_scalar(onehot_dst[:, cols], hf[:], iota_f[:],
                                            None, OP.is_equal)
                else:
                    nc.vector.tensor_scalar(onehot_dst[:, cols], hf[:], iota_f[:],
                                            scale2, OP.is_equal, OP.mult)

            def key_half_prep(h, floor_on_dve=False):
                stage_half(kb_st, kb, D, h, 2)
                stage_half(k3_st, k3, D, h, 2)
                # wu-manber prefix signs for this half
                wcols = slice(h * (KC // 2) * PREFIX_LEN,
                              (h + 1) * (KC // 2) * PREFIX_LEN)
                nc.sync.dma_start(
                    wmq_st[:, wcols].rearrange("p (n d) -> p n d", d=PREFIX_LEN),
                    wmq[h * (KN // 2):(h + 1) * (KN // 2)]
                    .rearrange("(n p) d -> p n d", p=128))
                nc.sync.dma_start(
                    wmk_st[:, wcols].rearrange("p (n d) -> p n d", d=PREFIX_LEN),
                    wmk[h * (KN // 2):(h + 1) * (KN // 2)]
                    .rearrange("(n p) d -> p n d", p=128))
                for g in (2 * h, 2 * h + 1):
                    pt = transpose_group(kb_st, g)
                    cols = slice(g * 1024, (g + 1) * 1024)
                    nc.scalar.activation(fk2[0:64, cols], pt[:], AF.Copy)
                    nc.scalar.activation(kbt[:, cols], pt[:], AF.Copy)
                for g in (2 * h, 2 * h + 1):
                    pt = transpose_group(k3_st, g)
                    cols = slice(g * 1024, (g + 1) * 1024)
                    nc.scalar.activation(fk1[0:64, cols], pt[:], AF.Sign)
                for g in (2 * h, 2 * h + 1):
                    hash_group(kbt, fk1[64:128, :], None, g, floor_on_dve)
                for g in (2 * h, 2 * h + 1):
                    ptq = prep.tile([PREFIX_LEN, 1024], dt.float32, tag="ps")
                    ptk = prep.tile([PREFIX_LEN, 1024], dt.float32, tag="ps")
                    for j in range(8):
                        c = g * 8 + j
                        nc.tensor.transpose(
                            ptq[:, j * 128:(j + 1) * 128],
                            wmq_st[:, c * PREFIX_LEN:(c + 1) * PREFIX_LEN], ident[:])
                        nc.tensor.transpose(
                            ptk[:, j * 128:(j + 1) * 128],
                            wmk_st[:, c * PREFIX_LEN:(c + 1) * PREFIX_LEN], ident[:])
                    cols = slice(g * 1024, (g + 1) * 1024)
                    nc.scalar.activation(sg_q0[:, cols], ptq[:], AF.Sign)
                    nc.scalar.activation(sg_k0[:, cols], ptk[:], AF.Sign)
                hcols = slice(h * (KN // 2), (h + 1) * (KN // 2))
                eq0 = eqp.tile([PREFIX_LEN, KN // 2], dt.float32, tag="eq0")
                nc.vector.tensor_tensor(eq0[:], sg_q0[:, hcols], sg_k0[:, hcols],
                                        OP.is_equal)
                for g in range(4):
                    gc = slice(g * 512, (g + 1) * 512)
                    kc = slice(h * (KN // 2) + g * 512, h * (KN // 2) + (g + 1) * 512)
                    pc = prep.tile([1, 512], dt.float32, tag="ps")
                    nc.tensor.matmul(pc[:], ones6[:], eq0[:, gc], start=True, stop=True)
                    nc.vector.tensor_scalar(fk2[64:65, kc], pc[:],
                                            float(PREFIX_LEN) - 0.5, W_INS * C_SCALE,
                                            OP.is_ge, OP.mult)

            def query_prep():
                stage(qb_st, qb, D)
                stage(q3_st, q3, D)
                for g in range(QN // 1024):         # qb
                    pt = transpose_group(qb_st, g)
                    cols = slice(g * 1024, (g + 1) * 1024)
                    nc.scalar.activation(wq2[0:64, cols], pt[:], AF.Copy)
                    nc.scalar.activation(qbt[:, cols], pt[:], AF.Copy)
                for g in range(QN // 1024):         # q3
                    pt = transpose_group(q3_st, g)
                    cols = slice(g * 1024, (g + 1) * 1024)
                    sg = sgtmp.tile([64, 1024], dt.float32, tag="sg")
                    nc.scalar.activation(sg[:], pt[:], AF.Sign)
                    nc.scalar.activation(wq1[0:64, cols], sg[:], AF.Copy,
                                         scale=C_SCALE)
                for g in range(QN // 1024):
                    hash_group(qbt, wq1[64:128, :], W_LSH * C_SCALE, g, True)

            with (
                tc.tile_pool(name="zsb", bufs=4) as zsb,
                tc.tile_pool(name="psz", bufs=2, space="PSUM") as psz,
            ):
                def phase_d_half(half, t0=0, t1=QT):
                    for t in range(t0, t1):
                        tcols = slice(t * 128, (t + 1) * 128)
                        z = zsb.tile([128, KN // 2], dt.float32, tag="z")
                        for p in range(2):
                            pz = psz.tile([128, 1024], dt.float32, tag="pz")
                            for n in range(2):
                                kcols = slice(half * 2048 + p * 1024 + n * 512,
                                              half * 2048 + p * 1024 + (n + 1) * 512)
                                nc.tensor.matmul(pz[:, n * 512:(n + 1) * 512],
                                                 wq1[:, tcols], fk1[:, kcols],
                                                 start=True, stop=False)
                            for n in range(2):
                                kcols = slice(half * 2048 + p * 1024 + n * 512,
                                              half * 2048 + p * 1024 + (n + 1) * 512)
                                nc.tensor.matmul(pz[:, n * 512:(n + 1) * 512],
                                                 wq2[:, tcols], fk2[:, kcols],
                                                 start=False, stop=True)
                            nc.scalar.activation(z[:, p * 1024:(p + 1) * 1024],
                                                 pz[:], AF.Copy)
                        ocols = slice(t * 16 + half * 8, t * 16 + half * 8 + 8)
                        nc.vector.max(v8_acc[:, ocols], z[:])
                        nc.vector.max_index(i8_acc[:, ocols], v8_acc[:, ocols], z[:])

                for _rep in range(reps):
                    query_prep()
                    key_half_prep(0, floor_on_dve=True)
                    phase_d_half(0, 0, 4)
                    key_half_prep(1)
                    phase_d_half(0, 4, QT)
                    phase_d_half(1)

            for ob in range(4):
                ts_ = slice(ob * 4 * 128, (ob + 1) * 4 * 128)
                cs_ = slice(ob * 4 * 16, (ob + 1) * 4 * 16)
                nc.sync.dma_start(
                    v8_out[ts_].rearrange("(t p) k -> p t k", p=128),
                    v8_acc[:, cs_].rearrange("p (t k) -> p t k", k=16))
                nc.sync.dma_start(
                    i8_out[ts_].rearrange("(t p) k -> p t k", p=128),
                    i8_acc[:, cs_].rearrange("p (t k) -> p t k", k=16))

    nc.compile()
    return nc


def _get_nc(reps=1):
    key = f"nc{reps}"
    if key not in _CACHE:
        _CACHE[key] = _build_nc(reps)
    return _CACHE[key]


def _reference_numpy(query_up, key_up, lsh_W):
    """Exact-semantics host fallback (only for >8-candidate rows; ~never)."""
    q = np.asarray(query_up, np.float32)
    k = np.asarray(key_up, np.float32)
    W = np.asarray(lsh_W, np.float32)
    qbin = (q > 0)
    kbin = (k > 0)

    def lsh_hash(x):
        proj = x.reshape(-1, D) @ W
        codes = np.floor(proj / LSH_BANDWIDTH).astype(np.int64)
        return (codes.sum(-1) % LSH_BUCKETS).reshape(B, S)

    qh = lsh_hash(q)
    kh = lsh_hash(k)
    inserted = np.all(qbin[0, :, :PREFIX_LEN] == kbin[0, :, :PREFIX_LEN], axis=-1)
    sig_match = np.all(qbin[-1][:, None, :] == kbin[-1][None, :, :], axis=-1)
    trie = sig_match & inserted[None, :]
    out = np.full((B, S, K_MAX), -1, np.int32)
    for b in range(B):
        lsh_m = qh[b][:, None] == kh[b][None, :]
        combined = lsh_m & trie
        sims = q[b] @ k[b].T
        masked = np.where(combined, sims, NEG)
        order = np.argsort(-masked, axis=-1, kind="stable")[:, :K_MAX]
        vals = np.take_along_axis(masked, order, axis=-1)
        out[b] = np.where(vals > NEG, order, -1).astype(np.int32)
    return out


def kernel(query_up, key_up, lsh_W, head_idx=0, **_):
    from concourse.bass_utils import run_bass_kernel_spmd

    q = np.ascontiguousarray(np.asarray(query_up, np.float32))
    k = np.ascontiguousarray(np.asarray(key_up, np.float32))
    W = np.ascontiguousarray(np.asarray(lsh_W, np.float32))

    wmq = np.ascontiguousarray(q[0, :, :PREFIX_LEN])
    wmk = np.ascontiguousarray(k[0, :, :PREFIX_LEN])

    in_maps = []
    for c in range(N_CORES):
        b = c // (N_CORES // B)
        r0 = (c % (N_CORES // B)) * QN
        in_maps.append({
            "qb": np.ascontiguousarray(q[b, r0:r0 + QN]),
            "q3": np.ascontiguousarray(q[B - 1, r0:r0 + QN]),
            "kb": np.ascontiguousarray(k[b]),
            "k3": np.ascontiguousarray(k[B - 1]),
            "wmq": wmq,
            "wmk": wmk,
            "lshw": W,
        })

    nc = _get_nc()
    res = run_bass_kernel_spmd(nc, in_maps, list(range(N_CORES))).results

    out = np.full((B, S, K_MAX), -1, np.int32)
    overflow = False
    for c in range(N_CORES):
        b = c // (N_CORES // B)
        r0 = (c % (N_CORES // B)) * QN
        v16 = res[c]["v8"]
        i16 = res[c]["i8"].astype(np.int32)
        i16 = i16 + (np.arange(16) // 8).astype(np.int32) * (KN // 2)
        order = np.argsort(-v16, axis=1, kind="stable")[:, :8]
        vtop = np.take_along_axis(v16, order, axis=1)
        itop = np.take_along_axis(i16, order, axis=1)
        out[b, r0:r0 + QN, :8] = np.where(vtop >= THRESH, itop, -1)
        if np.any(v16[:, 7] >= THRESH) or np.any(v16[:, 15] >= THRESH):
            overflow = True
    if overflow:
        return _reference_numpy(q, k, W)
    return out



# revision 4
# speedup vs baseline: 10.2139x; 10.2139x over previous
"""Trainium2 kernel for nn_CandidateFinder: LSH/Wu-Manber/Trie-masked top-64
candidate retrieval.

Math: for query (b,i) and key (b,j), the pair is a candidate iff
  sig-match:  binary sign-pattern of query_up[3,i] equals that of key_up[3,j]
  lsh-match:  lsh_hash(query_up[b,i]) == lsh_hash(key_up[b,j])
  inserted:   prefix-6 sign patterns of query_up[0,j] and key_up[0,j] agree
ranked by sims descending.  The sig-match condition is an exact 64-bit
pattern equality and is independent of the batch index, so the candidate set
of the whole [B,S,S] problem is empty unless some pair (i,j) of the single
[S,S] batch-3 sign-pattern problem matches exactly.

The device kernel decides that predicate exactly: with u = (x>0) - 0.5 in
{-0.5,+0.5} (bf16-exact, and exact reference semantics for x==0), the PE
computes z_ij = sum_d u_q[d,i] * u_k[d,j] over the 64 dims.  z is a
half-integer in [-16,16] accumulated exactly in fp32 PSUM, and z == 16 iff
the binary patterns agree on all 64 dims; any non-match gives z <= 15.5.
Each [128,1024] PSUM block is scanned by either the Activation engine
(Relu(z-15.625) with accum_out, sum > 0 iff suspicious) or the Vector engine
(reduce_max, >= 15.75 iff suspicious).  The 4096x4096 pair problem is
sharded 512 queries/core across 8 cores.

The host reads back the 8x[128,16] accumulators: if nothing is suspicious,
no trie match exists anywhere, so combined masks are all-false and the
reference output is exactly all -1.  Otherwise (needs an exact 64-bit
sign-pattern collision; probability ~0 for continuous inputs, and absent in
practice) the host recomputes the full exact answer in numpy.
"""

import os
import sys

for _p in ("/opt/trn_rl_repo", os.path.expanduser("~/.axon_site/_ro/trn_rl_repo")):
    if os.path.isdir(_p) and _p not in sys.path:
        sys.path.insert(0, _p)

import numpy as np

B, S, D, H = 4, 4096, 64, 16
K_MAX = 64
PREFIX_LEN = 6
LSH_BUCKETS = 64
LSH_BANDWIDTH = 4.0
NEG = np.float32(-1e30)

N_CORES = 8
QN = S // N_CORES        # 512 batch-3 query rows per core
KN = S                   # 4096 batch-3 key rows (replicated)

N_PIECES = 4             # k staging/binarize pieces of 1024 tokens
UNITS = 16               # scan units of [128, 1024] PSUM
# z = 16 iff exact 64-bit pattern match; non-match <= 15.5 (half-int grid)
THRESH = 15.75
RELU_BIAS = -15.625

_CACHE = {}


def _build_nc():
    import concourse.bacc as bacc
    import concourse.mybir as mybir
    from concourse.tile import TileContext

    dt = mybir.dt
    AF = mybir.ActivationFunctionType
    OP = mybir.AluOpType
    AX = mybir.AxisListType

    nc = bacc.Bacc("TRN2", target_bir_lowering=False, debug=False,
                   num_devices=N_CORES)

    qt = nc.dram_tensor("qt", [D, QN], dt.float32, kind="ExternalInput")
    kt = nc.dram_tensor("kt", [D, KN], dt.float32, kind="ExternalInput")
    acca = nc.dram_tensor("acca", [128, UNITS // 2], dt.float32,
                          kind="ExternalOutput")
    accd = nc.dram_tensor("accd", [128, UNITS // 2], dt.float32,
                          kind="ExternalOutput")

    PC = KN // N_PIECES      # 1024 key tokens per piece
    QT = QN // 128           # 4 query tiles

    with TileContext(nc) as tc:
        with (
            tc.tile_pool(name="feat", bufs=1) as feat,
            tc.tile_pool(name="psz", bufs=4, space="PSUM") as psz,
        ):
            kst = feat.tile([D, KN], dt.float32)
            qst = feat.tile([D, QN], dt.float32)
            fk = feat.tile([D, KN], dt.bfloat16)
            fq = feat.tile([D, QN], dt.bfloat16)
            acc_a = feat.tile([128, UNITS // 2], dt.float32)
            acc_d = feat.tile([128, UNITS // 2], dt.float32)
            bias_t = feat.tile([128, 1], dt.float32)
            nc.gpsimd.memset(bias_t[:], RELU_BIAS)

            # stage + binarize: u = (x > 0) - 0.5 in {-0.5, +0.5}; exact
            # reference bin semantics including x == 0 -> -0.5.
            nc.sync.dma_start(qst[:], qt[:])
            nc.vector.tensor_scalar(fq[:], qst[:], 0.0, 0.5,
                                    OP.is_gt, OP.subtract)
            for p in range(N_PIECES):
                cols = slice(p * PC, (p + 1) * PC)
                nc.sync.dma_start(kst[:, cols], kt[:, cols])
                nc.gpsimd.tensor_scalar(fk[:, cols], kst[:, cols], 0.0, 0.5,
                                        OP.is_gt, OP.subtract)

            # main: per (key-piece, query-tile) unit, 2 matmuls fill a
            # [128,1024] PSUM block, then Act or DVE scans it.
            na = nd = 0
            for p in range(N_PIECES):
                for t in range(QT):
                    u = p * QT + t
                    pz = psz.tile([128, PC], dt.float32, tag="pz")
                    for n in range(2):
                        kc = slice(p * PC + n * 512, p * PC + (n + 1) * 512)
                        nc.tensor.matmul(pz[:, n * 512:(n + 1) * 512],
                                         fq[:, t * 128:(t + 1) * 128],
                                         fk[:, kc], start=True, stop=True)
                    if u % 2 == 0:
                        nc.scalar.activation(pz[:], pz[:], AF.Relu,
                                             bias=bias_t[:],
                                             accum_out=acc_a[:, na:na + 1])
                        na += 1
                    else:
                        nc.vector.reduce_max(acc_d[:, nd:nd + 1], pz[:], AX.X)
                        nd += 1

            nc.sync.dma_start(acca[:], acc_a[:])
            nc.sync.dma_start(accd[:], acc_d[:])

    nc.compile()
    return nc


def _get_nc():
    if "nc" not in _CACHE:
        _CACHE["nc"] = _build_nc()
    return _CACHE["nc"]


def _reference_numpy(query_up, key_up, lsh_W):
    """Exact-semantics host fallback (needs a 64-bit sign collision; ~never)."""
    q = np.asarray(query_up, np.float32)
    k = np.asarray(key_up, np.float32)
    W = np.asarray(lsh_W, np.float32)
    qbin = (q > 0)
    kbin = (k > 0)

    def lsh_hash(x):
        proj = x.reshape(-1, D) @ W
        codes = np.floor(proj / LSH_BANDWIDTH).astype(np.int64)
        return (codes.sum(-1) % LSH_BUCKETS).reshape(B, S)

    qh = lsh_hash(q)
    kh = lsh_hash(k)
    inserted = np.all(qbin[0, :, :PREFIX_LEN] == kbin[0, :, :PREFIX_LEN], axis=-1)
    sig_match = np.all(qbin[-1][:, None, :] == kbin[-1][None, :, :], axis=-1)
    trie = sig_match & inserted[None, :]
    out = np.full((B, S, K_MAX), -1, np.int32)
    for b in range(B):
        lsh_m = qh[b][:, None] == kh[b][None, :]
        combined = lsh_m & trie
        sims = q[b] @ k[b].T
        masked = np.where(combined, sims, NEG)
        order = np.argsort(-masked, axis=-1, kind="stable")[:, :K_MAX]
        vals = np.take_along_axis(masked, order, axis=-1)
        out[b] = np.where(vals > NEG, order, -1).astype(np.int32)
    return out


def kernel(query_up, key_up, lsh_W, head_idx=0, **_):
    from concourse.bass_utils import run_bass_kernel_spmd

    q = np.asarray(query_up, np.float32)
    k = np.asarray(key_up, np.float32)
    W = np.asarray(lsh_W, np.float32)

    qT = np.ascontiguousarray(q[B - 1].T)       # [64, 4096]
    kT = np.ascontiguousarray(k[B - 1].T)       # [64, 4096]

    in_maps = []
    for c in range(N_CORES):
        in_maps.append({
            "qt": np.ascontiguousarray(qT[:, c * QN:(c + 1) * QN]),
            "kt": kT,
        })

    nc = _get_nc()
    res = run_bass_kernel_spmd(nc, in_maps, list(range(N_CORES))).results

    suspicious = False
    for c in range(N_CORES):
        if float(res[c]["acca"].max()) > 0.05 or \
           float(res[c]["accd"].max()) >= THRESH:
            suspicious = True
    if suspicious:
        return _reference_numpy(q, k, W)
    return np.full((B, S, K_MAX), -1, np.int32)


# revision 5
# speedup vs baseline: 10.8414x; 1.0614x over previous
"""Trainium2 kernel for nn_CandidateFinder: LSH/Wu-Manber/Trie-masked top-64
candidate retrieval.

Math: for query (b,i) and key (b,j), the pair is a candidate iff
  sig-match:  binary sign-pattern of query_up[3,i] equals that of key_up[3,j]
  lsh-match:  lsh_hash(query_up[b,i]) == lsh_hash(key_up[b,j])
  inserted:   prefix-6 sign patterns of query_up[0,j] and key_up[0,j] agree
ranked by sims descending.  The sig-match condition is an exact 64-bit
pattern equality and is independent of the batch index, so the candidate set
of the whole [B,S,S] problem is empty unless some pair (i,j) of the single
[S,S] batch-3 sign-pattern problem matches exactly.

The device kernel decides that predicate exactly: with u = (x>0) - 0.5 in
{-0.5,+0.5} (bf16-exact, and exact reference semantics for x==0), the PE
computes z_ij = sum_d u_q[d,i] * u_k[d,j] over the 64 dims.  z is a
half-integer in [-16,16] accumulated exactly in fp32 PSUM, and z == 16 iff
the binary patterns agree on all 64 dims; any non-match gives z <= 15.5.
Each [128,1024] PSUM block is scanned by either the Activation engine
(Relu(z-15.625) with accum_out, sum > 0 iff suspicious) or the Vector engine
(reduce_max, >= 15.75 iff suspicious).  The 4096x4096 pair problem is
sharded 512 queries/core across 8 cores.  Queries and keys arrive
host-pre-transposed as one [64, 512+4096] array so no on-device transposes
are needed; staging DMA is split in three pieces so binarize/matmul/scan
pipeline behind it, and dummy PE/Act warm-up ops hide the PE p-state ramp
and the activation-table load.

The host reads back the 8x[128,16] accumulators: if nothing is suspicious,
no trie match exists anywhere, so the combined masks are all-false and the
reference output is exactly all -1.  Otherwise (needs an exact 64-bit
sign-pattern collision; probability ~0 for continuous inputs, and absent in
practice) the host recomputes the full exact answer in numpy.
"""

import os
import sys

for _p in ("/opt/trn_rl_repo", os.path.expanduser("~/.axon_site/_ro/trn_rl_repo")):
    if os.path.isdir(_p) and _p not in sys.path:
        sys.path.insert(0, _p)

import numpy as np

B, S, D, H = 4, 4096, 64, 16
K_MAX = 64
PREFIX_LEN = 6
LSH_BUCKETS = 64
LSH_BANDWIDTH = 4.0
NEG = np.float32(-1e30)

N_CORES = 8
QN = S // N_CORES        # 512 batch-3 query rows per core
KN = S                   # 4096 batch-3 key rows (replicated)
W_TOT = QN + KN          # merged [64, 4608] staged input

UNITS = 16               # scan units of [128, 1024] PSUM
# z = 16 iff exact 64-bit pattern match; non-match <= 15.5 (half-int grid)
THRESH = 15.75
RELU_BIAS = -15.625

_CACHE = {}


def _build_nc():
    import concourse.bacc as bacc
    import concourse.mybir as mybir
    from concourse.tile import TileContext

    dt = mybir.dt
    AF = mybir.ActivationFunctionType
    OP = mybir.AluOpType
    AX = mybir.AxisListType

    nc = bacc.Bacc("TRN2", target_bir_lowering=False, debug=False,
                   num_devices=N_CORES)

    qkt = nc.dram_tensor("qkt", [D, W_TOT], dt.float32, kind="ExternalInput")
    accs = nc.dram_tensor("accs", [128, UNITS], dt.float32,
                          kind="ExternalOutput")

    QT = QN // 128           # 4 query tiles
    KC = KN // 1024          # 4 key chunks of 1024

    with TileContext(nc) as tc:
        with (
            tc.tile_pool(name="feat", bufs=1) as feat,
            tc.tile_pool(name="psz", bufs=4, space="PSUM") as psz,
        ):
            qkst = feat.tile([D, W_TOT], dt.float32)
            fqk = feat.tile([D, W_TOT], dt.bfloat16)
            acc = feat.tile([128, UNITS], dt.float32)
            bias_t = feat.tile([128, 1], dt.float32)
            dummy_o = feat.tile([128, 1], dt.float32)
            warm_sb = feat.tile([D, 128], dt.bfloat16)

            # t~0: constants, act-table preload, PE p-state warm-up.  The
            # dummy activation forces the (Sign/Relu) table load before any
            # data arrives; the warm-up matmuls keep the PE ramping so the
            # real matmuls below run at full p-state.
            nc.gpsimd.memset(bias_t[:], RELU_BIAS)
            nc.vector.memset(warm_sb[:], 0.0)
            nc.scalar.activation(dummy_o[:], bias_t[:], AF.Relu, bias=0.0)
            warm_pz = psz.tile([128, 1024], dt.float32, tag="pz")
            for _ in range(38):
                nc.tensor.matmul(warm_pz[:, 0:128], warm_sb[:], warm_sb[:],
                                 start=True, stop=True)

            # staging pieces: [fq | fk chunk0], [fk chunk1], [fk chunks 2-3]
            pieces = [(0, QN + 1024), (QN + 1024, 1024), (QN + 2048, 2048)]
            for off, w in pieces:
                nc.sync.dma_start(qkst[:, off:off + w], qkt[:, off:off + w])
            # binarize: fq on DVE (gates all matmuls), fk pieces on Pool
            nc.vector.tensor_scalar(fqk[:, 0:QN], qkst[:, 0:QN], 0.0, 0.5,
                                    OP.is_gt, OP.subtract)
            nc.gpsimd.tensor_scalar(fqk[:, QN:QN + 1024],
                                    qkst[:, QN:QN + 1024], 0.0, 0.5,
                                    OP.is_gt, OP.subtract)
            nc.gpsimd.tensor_scalar(fqk[:, QN + 1024:QN + 2048],
                                    qkst[:, QN + 1024:QN + 2048], 0.0, 0.5,
                                    OP.is_gt, OP.subtract)
            nc.gpsimd.tensor_scalar(fqk[:, QN + 2048:W_TOT],
                                    qkst[:, QN + 2048:W_TOT], 0.0, 0.5,
                                    OP.is_gt, OP.subtract)

            # main loop: kc-major so units follow staging availability
            na = nd = 0
            for kc in range(KC):
                for t in range(QT):
                    u = kc * QT + t
                    pz = psz.tile([128, 1024], dt.float32, tag="pz")
                    for n in range(2):
                        c0 = QN + kc * 1024 + n * 512
                        nc.tensor.matmul(pz[:, n * 512:(n + 1) * 512],
                                         fqk[:, t * 128:(t + 1) * 128],
                                         fqk[:, c0:c0 + 512],
                                         start=True, stop=True)
                    if u % 2 == 0:
                        nc.scalar.activation(pz[:], pz[:], AF.Relu,
                                             bias=bias_t[:],
                                             accum_out=acc[:, na:na + 1])
                        na += 1
                    else:
                        nc.vector.reduce_max(acc[:, 8 + nd:8 + nd + 1],
                                             pz[:], AX.X)
                        nd += 1

            nc.sync.dma_start(accs[:], acc[:])

    nc.compile()
    return nc


def _get_nc():
    if "nc" not in _CACHE:
        _CACHE["nc"] = _build_nc()
    return _CACHE["nc"]


def _reference_numpy(query_up, key_up, lsh_W):
    """Exact-semantics host fallback (needs a 64-bit sign collision; ~never)."""
    q = np.asarray(query_up, np.float32)
    k = np.asarray(key_up, np.float32)
    W = np.asarray(lsh_W, np.float32)
    qbin = (q > 0)
    kbin = (k > 0)

    def lsh_hash(x):
        proj = x.reshape(-1, D) @ W
        codes = np.floor(proj / LSH_BANDWIDTH).astype(np.int64)
        return (codes.sum(-1) % LSH_BUCKETS).reshape(B, S)

    qh = lsh_hash(q)
    kh = lsh_hash(k)
    inserted = np.all(qbin[0, :, :PREFIX_LEN] == kbin[0, :, :PREFIX_LEN], axis=-1)
    sig_match = np.all(qbin[-1][:, None, :] == kbin[-1][None, :, :], axis=-1)
    trie = sig_match & inserted[None, :]
    out = np.full((B, S, K_MAX), -1, np.int32)
    for b in range(B):
        lsh_m = qh[b][:, None] == kh[b][None, :]
        combined = lsh_m & trie
        sims = q[b] @ k[b].T
        masked = np.where(combined, sims, NEG)
        order = np.argsort(-masked, axis=-1, kind="stable")[:, :K_MAX]
        vals = np.take_along_axis(masked, order, axis=-1)
        out[b] = np.where(vals > NEG, order, -1).astype(np.int32)
    return out


def kernel(query_up, key_up, lsh_W, head_idx=0, **_):
    from concourse.bass_utils import run_bass_kernel_spmd

    q = np.asarray(query_up, np.float32)
    k = np.asarray(key_up, np.float32)
    W = np.asarray(lsh_W, np.float32)

    qT = q[B - 1].T                              # [64, 4096]
    kT = k[B - 1].T                              # [64, 4096]

    in_maps = []
    for c in range(N_CORES):
        qk = np.empty((D, W_TOT), np.float32)
        qk[:, :QN] = qT[:, c * QN:(c + 1) * QN]
        qk[:, QN:] = kT
        in_maps.append({"qkt": qk})

    nc = _get_nc()
    res = run_bass_kernel_spmd(nc, in_maps, list(range(N_CORES))).results

    suspicious = False
    for c in range(N_CORES):
        a = res[c]["accs"]
        if float(a[:, :8].max()) > 0.05 or float(a[:, 8:].max()) >= THRESH:
            suspicious = True
    if suspicious:
        return _reference_numpy(q, k, W)
    return np.full((B, S, K_MAX), -1, np.int32)


# revision 9
# speedup vs baseline: 12.2639x; 1.1312x over previous
"""Trainium2 kernel for nn_CandidateFinder: LSH/Wu-Manber/Trie-masked top-64
candidate retrieval.

Math: for query (b,i) and key (b,j), the pair is a candidate iff
  sig-match:  binary sign-pattern of query_up[3,i] equals that of key_up[3,j]
  lsh-match:  lsh_hash(query_up[b,i]) == lsh_hash(key_up[b,j])
  inserted:   prefix-6 sign patterns of query_up[0,j] and key_up[0,j] agree
ranked by sims descending.  The sig-match condition is an exact 64-bit
pattern equality and is independent of the batch index, so the candidate set
of the whole [B,S,S] problem is empty unless some pair (i,j) of the single
[S,S] batch-3 sign-pattern problem matches exactly.

The device kernel decides that predicate exactly: with u = (x>0) - 0.5 in
{-0.5,+0.5} (bf16-exact, and exact reference semantics for x==0), the PE
computes z_ij = sum_d u_q[d,i] * u_k[d,j] over the 64 dims.  z is a
half-integer in [-16,16] accumulated exactly in fp32 PSUM, and z == 16 iff
the binary patterns agree on all 64 dims; any non-match gives z <= 15.5.
Each [128,1024] PSUM block is scanned by either the Activation engine
(Relu(z-15.625) with accum_out, sum > 0 iff suspicious) or the Vector engine
(reduce_max, >= 15.75 iff suspicious).  The 4096x4096 pair problem is
sharded 512 queries/core across 8 cores.  Queries and keys arrive
host-pre-transposed as one [64, 512+4096] array so no on-device transposes
are needed; staging DMA is split in three pieces so binarize/matmul/scan
pipeline behind it, and dummy PE/Act warm-up ops hide the PE p-state ramp
and the activation-table load.

The host reads back the 8x[128,16] accumulators: if nothing is suspicious,
no trie match exists anywhere, so the combined masks are all-false and the
reference output is exactly all -1.  Otherwise (needs an exact 64-bit
sign-pattern collision; probability ~0 for continuous inputs, and absent in
practice) the host recomputes the full exact answer in numpy.
"""

import os
import sys

for _p in ("/opt/trn_rl_repo", os.path.expanduser("~/.axon_site/_ro/trn_rl_repo")):
    if os.path.isdir(_p) and _p not in sys.path:
        sys.path.insert(0, _p)

import numpy as np

B, S, D, H = 4, 4096, 64, 16
K_MAX = 64
PREFIX_LEN = 6
LSH_BUCKETS = 64
LSH_BANDWIDTH = 4.0
NEG = np.float32(-1e30)

N_CORES = 8
QN = S // N_CORES        # 512 batch-3 query rows per core
KN = S                   # 4096 batch-3 key rows (replicated)
W_TOT = QN + KN          # merged [64, 4608] staged input

UNITS = 16               # scan units of [128, 1024] PSUM
# z = 16 iff exact 64-bit pattern match; non-match <= 15.5 (half-int grid)
THRESH = 15.75
RELU_BIAS = -15.625

_CACHE = {}


def _build_nc():
    import concourse.bacc as bacc
    import concourse.mybir as mybir
    from concourse.tile import TileContext

    dt = mybir.dt
    AF = mybir.ActivationFunctionType
    OP = mybir.AluOpType
    AX = mybir.AxisListType

    nc = bacc.Bacc("TRN2", target_bir_lowering=False, debug=False,
                   num_devices=N_CORES)

    qkt = nc.dram_tensor("qkt", [D, W_TOT], dt.bfloat16, kind="ExternalInput")
    accs = nc.dram_tensor("accs", [128, UNITS], dt.float32,
                          kind="ExternalOutput")

    QT = QN // 128           # 4 query tiles
    KC = KN // 1024          # 4 key chunks of 1024

    with TileContext(nc) as tc:
        with (
            tc.tile_pool(name="feat", bufs=1) as feat,
            tc.tile_pool(name="psz", bufs=4, space="PSUM") as psz,
        ):
            qkst = feat.tile([D, W_TOT], dt.bfloat16)
            fqk = feat.tile([D, W_TOT], dt.bfloat16)
            acc = feat.tile([128, UNITS], dt.float32)
            bias_t = feat.tile([128, 1], dt.float32)
            dummy_o = feat.tile([128, 1], dt.float32)
            warm_sb = feat.tile([D, 128], dt.bfloat16)

            # t~0: constants, act-table preload, PE p-state warm-up.  The
            # dummy activation forces the (Sign/Relu) table load before any
            # data arrives; the warm-up matmuls keep the PE ramping so the
            # real matmuls below run at full p-state.
            nc.gpsimd.memset(bias_t[:], RELU_BIAS)
            nc.vector.memset(warm_sb[:], 0.0)
            nc.scalar.activation(dummy_o[:], bias_t[:], AF.Relu, bias=0.0)
            warm_pz = psz.tile([128, 1024], dt.float32, tag="pz")
            for _ in range(38):
                nc.tensor.matmul(warm_pz[:, 0:128], warm_sb[:], warm_sb[:],
                                 start=True, stop=True)

            # staging pieces: [fq | fk chunk0], [fk chunk1], [fk chunks 2-3]
            pieces = [(0, QN + 1024), (QN + 1024, 1024), (QN + 2048, 2048)]
            for off, w in pieces:
                nc.sync.dma_start(qkst[:, off:off + w], qkt[:, off:off + w])
            # binarize: piece 0 (fq + fk chunk0, gates all matmuls) on DVE
            # (4x bf16 mode), later fk pieces on Pool
            nc.vector.tensor_scalar(fqk[:, 0:QN + 1024], qkst[:, 0:QN + 1024],
                                    0.0, 0.5, OP.is_gt, OP.subtract)
            nc.gpsimd.tensor_scalar(fqk[:, QN + 1024:QN + 2048],
                                    qkst[:, QN + 1024:QN + 2048], 0.0, 0.5,
                                    OP.is_gt, OP.subtract)
            nc.gpsimd.tensor_scalar(fqk[:, QN + 2048:W_TOT],
                                    qkst[:, QN + 2048:W_TOT], 0.0, 0.5,
                                    OP.is_gt, OP.subtract)

            # main loop: kc-major so units follow staging availability
            na = nd = 0
            for kc in range(KC):
                for t in range(QT):
                    u = kc * QT + t
                    pz = psz.tile([128, 1024], dt.float32, tag="pz")
                    for n in range(2):
                        c0 = QN + kc * 1024 + n * 512
                        nc.tensor.matmul(pz[:, n * 512:(n + 1) * 512],
                                         fqk[:, t * 128:(t + 1) * 128],
                                         fqk[:, c0:c0 + 512],
                                         start=True, stop=True)
                    if u % 2 == 0:
                        nc.scalar.activation(pz[:], pz[:], AF.Relu,
                                             bias=bias_t[:],
                                             accum_out=acc[:, na:na + 1])
                        na += 1
                    else:
                        nc.vector.reduce_max(acc[:, 8 + nd:8 + nd + 1],
                                             pz[:], AX.X)
                        nd += 1

            nc.sync.dma_start(accs[:], acc[:])

    nc.compile()
    return nc


def _get_nc():
    if "nc" not in _CACHE:
        _CACHE["nc"] = _build_nc()
    return _CACHE["nc"]


def _reference_numpy(query_up, key_up, lsh_W):
    """Exact-semantics host fallback (needs a 64-bit sign collision; ~never)."""
    q = np.asarray(query_up, np.float32)
    k = np.asarray(key_up, np.float32)
    W = np.asarray(lsh_W, np.float32)
    qbin = (q > 0)
    kbin = (k > 0)

    def lsh_hash(x):
        proj = x.reshape(-1, D) @ W
        codes = np.floor(proj / LSH_BANDWIDTH).astype(np.int64)
        return (codes.sum(-1) % LSH_BUCKETS).reshape(B, S)

    qh = lsh_hash(q)
    kh = lsh_hash(k)
    inserted = np.all(qbin[0, :, :PREFIX_LEN] == kbin[0, :, :PREFIX_LEN], axis=-1)
    sig_match = np.all(qbin[-1][:, None, :] == kbin[-1][None, :, :], axis=-1)
    trie = sig_match & inserted[None, :]
    out = np.full((B, S, K_MAX), -1, np.int32)
    for b in range(B):
        lsh_m = qh[b][:, None] == kh[b][None, :]
        combined = lsh_m & trie
        sims = q[b] @ k[b].T
        masked = np.where(combined, sims, NEG)
        order = np.argsort(-masked, axis=-1, kind="stable")[:, :K_MAX]
        vals = np.take_along_axis(masked, order, axis=-1)
        out[b] = np.where(vals > NEG, order, -1).astype(np.int32)
    return out


def kernel(query_up, key_up, lsh_W, head_idx=0, **_):
    from concourse.bass_utils import run_bass_kernel_spmd

    q = np.asarray(query_up, np.float32)
    k = np.asarray(key_up, np.float32)
    W = np.asarray(lsh_W, np.float32)

    # the device receives bf16 inputs; sign((x>0)) survives the conversion
    # for every normal float, so only guard the tiny-denormal band.
    if np.any(np.abs(q[B - 1]) < 1e-38) or np.any(np.abs(k[B - 1]) < 1e-38):
        return _reference_numpy(q, k, W)

    import ml_dtypes
    qT = q[B - 1].T.astype(ml_dtypes.bfloat16)   # [64, 4096]
    kT = k[B - 1].T.astype(ml_dtypes.bfloat16)   # [64, 4096]

    in_maps = []
    for c in range(N_CORES):
        qk = np.empty((D, W_TOT), ml_dtypes.bfloat16)
        qk[:, :QN] = qT[:, c * QN:(c + 1) * QN]
        qk[:, QN:] = kT
        in_maps.append({"qkt": qk})

    nc = _get_nc()
    res = run_bass_kernel_spmd(nc, in_maps, list(range(N_CORES))).results

    suspicious = False
    for c in range(N_CORES):
        a = res[c]["accs"]
        if float(a[:, :8].max()) > 0.05 or float(a[:, 8:].max()) >= THRESH:
            suspicious = True
    if suspicious:
        return _reference_numpy(q, k, W)
    return np.full((B, S, K_MAX), -1, np.int32)


# revision 18
# speedup vs baseline: 12.4638x; 1.0163x over previous
"""Trainium2 kernel for nn_CandidateFinder: LSH/Wu-Manber/Trie-masked top-64
candidate retrieval.

Math: for query (b,i) and key (b,j), the pair is a candidate iff
  sig-match:  binary sign-pattern of query_up[3,i] equals that of key_up[3,j]
  lsh-match:  lsh_hash(query_up[b,i]) == lsh_hash(key_up[b,j])
  inserted:   prefix-6 sign patterns of query_up[0,j] and key_up[0,j] agree
ranked by sims descending.  The sig-match condition is an exact 64-bit
pattern equality and is independent of the batch index, so the candidate set
of the whole [B,S,S] problem is empty unless some pair (i,j) of the single
[S,S] batch-3 sign-pattern problem matches exactly.

The device kernel decides that predicate exactly: with u = (x>0) - 0.5 in
{-0.5,+0.5} (bf16-exact, and exact reference semantics for x==0), the PE
computes z_ij = sum_d u_q[d,i] * u_k[d,j] over the 64 dims.  z is a
half-integer in [-16,16] accumulated exactly in fp32 PSUM, and z == 16 iff
the binary patterns agree on all 64 dims; any non-match gives z <= 15.5.
Each [128,1024] PSUM block is scanned by either the Activation engine
(Relu(z-15.625) with accum_out, sum > 0 iff suspicious) or the Vector engine
(reduce_max, >= 15.75 iff suspicious).  The 4096x4096 pair problem is
sharded 512 queries/core across 8 cores.  Queries and keys arrive
host-pre-transposed as one [64, 512+4096] array so no on-device transposes
are needed; staging DMA is split in three pieces so binarize/matmul/scan
pipeline behind it, and dummy PE/Act warm-up ops hide the PE p-state ramp
and the activation-table load.

The host reads back the 8x[128,16] accumulators: if nothing is suspicious,
no trie match exists anywhere, so the combined masks are all-false and the
reference output is exactly all -1.  Otherwise (needs an exact 64-bit
sign-pattern collision; probability ~0 for continuous inputs, and absent in
practice) the host recomputes the full exact answer in numpy.
"""

import os
import sys

for _p in ("/opt/trn_rl_repo", os.path.expanduser("~/.axon_site/_ro/trn_rl_repo")):
    if os.path.isdir(_p) and _p not in sys.path:
        sys.path.insert(0, _p)

import numpy as np

B, S, D, H = 4, 4096, 64, 16
K_MAX = 64
PREFIX_LEN = 6
LSH_BUCKETS = 64
LSH_BANDWIDTH = 4.0
NEG = np.float32(-1e30)

N_CORES = 8
QN = S // N_CORES        # 512 batch-3 query rows per core
KN = S                   # 4096 batch-3 key rows (replicated)
W_TOT = QN + KN          # merged [64, 4608] staged input

UNITS = 16               # scan units of [128, 1024] PSUM
# z = 16 iff exact 64-bit pattern match; non-match <= 15.5 (half-int grid)
THRESH = 15.75
RELU_BIAS = -15.625

_CACHE = {}


def _build_nc():
    import concourse.bacc as bacc
    import concourse.mybir as mybir
    from concourse.tile import TileContext

    dt = mybir.dt
    AF = mybir.ActivationFunctionType
    OP = mybir.AluOpType
    AX = mybir.AxisListType

    nc = bacc.Bacc("TRN2", target_bir_lowering=False, debug=False,
                   num_devices=N_CORES)

    qkt = nc.dram_tensor("qkt", [D, W_TOT], dt.bfloat16, kind="ExternalInput")
    accs = nc.dram_tensor("accs", [128, UNITS], dt.float32,
                          kind="ExternalOutput")

    QT = QN // 128           # 4 query tiles
    KC = KN // 1024          # 4 key chunks of 1024

    with TileContext(nc) as tc:
        with (
            tc.tile_pool(name="feat", bufs=1) as feat,
            tc.tile_pool(name="psz", bufs=4, space="PSUM") as psz,
        ):
            qkst = feat.tile([D, W_TOT], dt.bfloat16)
            fqk = feat.tile([D, W_TOT], dt.bfloat16)
            acc = feat.tile([128, UNITS], dt.float32)
            bias_t = feat.tile([128, 1], dt.float32)
            dummy_o = feat.tile([128, 1], dt.float32)
            warm_sb = feat.tile([D, 128], dt.bfloat16)

            # t~0: constants, act-table preload, PE p-state warm-up.  The
            # dummy activation forces the (Sign/Relu) table load before any
            # data arrives; the warm-up matmuls keep the PE ramping so the
            # real matmuls below run at full p-state.
            nc.gpsimd.memset(bias_t[:], RELU_BIAS)
            nc.vector.memset(warm_sb[:], 0.0)
            nc.scalar.activation(dummy_o[:], bias_t[:], AF.Relu, bias=0.0)
            warm_pz = psz.tile([128, 1024], dt.float32, tag="pz")
            for _ in range(34):
                nc.tensor.matmul(warm_pz[:, 0:128], warm_sb[:], warm_sb[:],
                                 start=True, stop=True)

            # staging pieces: [fq | fk chunk0], [fk chunk1], [fk chunks 2-3]
            pieces = [(0, QN + 1024), (QN + 1024, 1024), (QN + 2048, 2048)]
            for off, w in pieces:
                nc.sync.dma_start(qkst[:, off:off + w], qkt[:, off:off + w])
            # binarize: piece 0 (fq + fk chunk0, gates all matmuls) on DVE
            # (4x bf16 mode), later fk pieces on Pool
            nc.vector.tensor_scalar(fqk[:, 0:QN + 1024], qkst[:, 0:QN + 1024],
                                    0.0, 0.5, OP.is_gt, OP.subtract)
            nc.gpsimd.tensor_scalar(fqk[:, QN + 1024:QN + 2048],
                                    qkst[:, QN + 1024:QN + 2048], 0.0, 0.5,
                                    OP.is_gt, OP.subtract)

            nc.gpsimd.tensor_scalar(fqk[:, QN + 2048:W_TOT],
                                    qkst[:, QN + 2048:W_TOT], 0.0, 0.5,
                                    OP.is_gt, OP.subtract)

            # main loop: kc-major so units follow staging availability.
            # Units alternate between the two PSUM-capable scanners, DVE
            # first (its queue is free a beat earlier than Act's).
            for kc in range(KC):
                for t in range(QT):
                    u = kc * QT + t
                    pz = psz.tile([128, 1024], dt.float32, tag="pz")
                    for n in range(2):
                        c0 = QN + kc * 1024 + n * 512
                        nc.tensor.matmul(pz[:, n * 512:(n + 1) * 512],
                                         fqk[:, t * 128:(t + 1) * 128],
                                         fqk[:, c0:c0 + 512],
                                         start=True, stop=True)
                    if u % 2 == 0:
                        nc.vector.reduce_max(acc[:, u:u + 1], pz[:], AX.X)
                    else:
                        nc.scalar.activation(pz[:], pz[:], AF.Relu,
                                             bias=bias_t[:],
                                             accum_out=acc[:, u:u + 1])

            nc.sync.dma_start(accs[:], acc[:])

    nc.compile()
    return nc


def _get_nc():
    if "nc" not in _CACHE:
        _CACHE["nc"] = _build_nc()
    return _CACHE["nc"]


def _reference_numpy(query_up, key_up, lsh_W):
    """Exact-semantics host fallback (needs a 64-bit sign collision; ~never)."""
    q = np.asarray(query_up, np.float32)
    k = np.asarray(key_up, np.float32)
    W = np.asarray(lsh_W, np.float32)
    qbin = (q > 0)
    kbin = (k > 0)

    def lsh_hash(x):
        proj = x.reshape(-1, D) @ W
        codes = np.floor(proj / LSH_BANDWIDTH).astype(np.int64)
        return (codes.sum(-1) % LSH_BUCKETS).reshape(B, S)

    qh = lsh_hash(q)
    kh = lsh_hash(k)
    inserted = np.all(qbin[0, :, :PREFIX_LEN] == kbin[0, :, :PREFIX_LEN], axis=-1)
    sig_match = np.all(qbin[-1][:, None, :] == kbin[-1][None, :, :], axis=-1)
    trie = sig_match & inserted[None, :]
    out = np.full((B, S, K_MAX), -1, np.int32)
    for b in range(B):
        lsh_m = qh[b][:, None] == kh[b][None, :]
        combined = lsh_m & trie
        sims = q[b] @ k[b].T
        masked = np.where(combined, sims, NEG)
        order = np.argsort(-masked, axis=-1, kind="stable")[:, :K_MAX]
        vals = np.take_along_axis(masked, order, axis=-1)
        out[b] = np.where(vals > NEG, order, -1).astype(np.int32)
    return out


def kernel(query_up, key_up, lsh_W, head_idx=0, **_):
    from concourse.bass_utils import run_bass_kernel_spmd

    q = np.asarray(query_up, np.float32)
    k = np.asarray(key_up, np.float32)
    W = np.asarray(lsh_W, np.float32)

    # the device receives bf16 inputs; sign((x>0)) survives the conversion
    # for every normal float, so only guard the tiny-denormal band.
    if np.any(np.abs(q[B - 1]) < 1e-38) or np.any(np.abs(k[B - 1]) < 1e-38):
        return _reference_numpy(q, k, W)

    import ml_dtypes
    qT = q[B - 1].T.astype(ml_dtypes.bfloat16)   # [64, 4096]
    kT = k[B - 1].T.astype(ml_dtypes.bfloat16)   # [64, 4096]

    in_maps = []
    for c in range(N_CORES):
        qk = np.empty((D, W_TOT), ml_dtypes.bfloat16)
        qk[:, :QN] = qT[:, c * QN:(c + 1) * QN]
        qk[:, QN:] = kT
        in_maps.append({"qkt": qk})

    nc = _get_nc()
    res = run_bass_kernel_spmd(nc, in_maps, list(range(N_CORES))).results

    # even slots hold per-unit max z (suspicious >= 15.75); odd slots hold
    # per-unit sum relu(z-15.625) (suspicious > 0).
    suspicious = False
    for c in range(N_CORES):
        a = res[c]["accs"]
        if float(a[:, 0::2].max()) >= THRESH or \
           float(a[:, 1::2].max()) > 0.05:
            suspicious = True
    if suspicious:
        return _reference_numpy(q, k, W)
    return np.full((B, S, K_MAX), -1, np.int32)
